# revision 4
# baseline (speedup 1.0000x reference)
"""Trainium2 Bass kernel for nn_DecoderLayer (Performer/FAVOR+ decoder layer).

Problem: B=4, N=6400, D=256, H=8, HD=32, DFF=1024, M(feat)=8.
  out = LN3(FFN(LN2(CrossPerf(LN1(SelfPerf(x)+x)) + ...)) + ...)

v2 design (no collectives, no DMA transposes, single act-table set):
  - 8 cores; core c handles batch c//2, token half c%2 (3200 own tokens).
  - kv (the token reduction of linear attention) is computed over the FULL
    6400 tokens per core (x / enc of its batch), so no AllReduce is needed.
    Host permutes tokens so each core's own half comes first; kv is
    order-invariant, the q side just reads tokens [0:3200].
  - kv accumulator [64, 257] f32 in PSUM: cols 0:256 = per-head v (h-major),
    col 256 = sum_t phi(k).  The block-diagonal attention operands are then
    plain slices kv[32g:32g+32, 128g:128g+128] -- no scatter, no DRAM trip.
  - Layout transposes (out1/out2 natural -> feature-major for the next
    matmul) are PE transposes via an identity matrix, not DMA transposes.
  - LayerNorm rstd = exp(-0.5*ln(var+eps)): ln/exp/square/copy all live in
    one activation-function table set, so no act-table reloads.
All matmul I/O bf16 (fp32 PSUM accumulation); residual stream bf16; final
LN3 output written f32.
"""

import numpy as np
import ml_dtypes

B, N, D, H, HD, DFF, M = 4, 6400, 256, 8, 32, 1024, 8
NCORES = 8
S = B * N // NCORES          # 3200 own tokens per core
EPS_LN = 1e-6
SCALE = float(HD) ** -0.25   # split softmax scale on q and k
HM = H * M                   # 64
VE = 257                     # kv cols: 256 v (h-major) + 1 pksum col
KP0 = 256                    # kp feature region start in v_ps psum

CH_FULL = [(i * 512, 512) for i in range(12)] + [(6144, 256)]
CH_OWN = [(i * 512, 512) for i in range(6)] + [(3072, 128)]

_BF = ml_dtypes.bfloat16


def _bf(a):
    return np.ascontiguousarray(np.asarray(a, np.float32)).astype(_BF)


# --------------------------------------------------------------------------
# Host-side weight packing
# --------------------------------------------------------------------------

def _pack_performer(qw, qb, kw, kb, vw, vb, ow, ob, feat):
    """Pack one performer's weights into the device layouts (all bf16)."""
    qw = np.asarray(qw, np.float32); kw = np.asarray(kw, np.float32)
    vw = np.asarray(vw, np.float32); ow = np.asarray(ow, np.float32)
    feat = np.asarray(feat, np.float32)
    assert np.allclose(np.asarray(qb), 0) and np.allclose(np.asarray(kb), 0), \
        "nonzero q/k bias not supported by this kernel build"
    assert np.allclose(np.asarray(vb), 0) and np.allclose(np.asarray(ob), 0), \
        "nonzero v/o bias not supported by this kernel build"

    Wq = (SCALE * qw).reshape(D, D)          # [d, (h,hd)]
    Wk = (SCALE * kw).reshape(D, D)
    # W_qp[d, 8h+j] = sum_hd s*qw[d,h,hd] * feat[hd,j]
    Wqp = np.einsum('dhk,km->dhm', SCALE * qw, feat).reshape(D, HM)
    Wkp = np.einsum('dhk,km->dhm', SCALE * kw, feat).reshape(D, HM)

    # blockdiag -0.5 reducer over a 128-row douttile (4 heads x 32 hd)
    negblk = np.zeros((2, 128, HM), np.float32)
    for g in range(2):
        for lh in range(4):
            h = 4 * g + lh
            negblk[g, 32 * lh:32 * lh + 32, 8 * h:8 * h + 8] = -0.5

    # vkw [256, 320]: cols 0:256 v-projection (h-major), 256:320 Wkp
    vkw = np.concatenate([vw.reshape(D, D), Wkp], axis=1)

    Wo = np.zeros((2, 128, D), np.float32)   # out-proj rhs per group g
    for g in range(2):
        for lh in range(4):
            h = 4 * g + lh
            Wo[g, 32 * lh:32 * lh + 32, :] = ow[h, :, :]

    mask8 = np.zeros((HM, HM), np.float32)   # [8h+j, 8h'+j'] = [h==h']
    for h in range(H):
        mask8[8*h:8*h+8, 8*h:8*h+8] = 1.0

    def t2(a):  # [D, C] -> [128, 2, C] with [i, kt, c] = a[128*kt + i, c]
        return _bf(a.reshape(2, 128, -1).transpose(1, 0, 2))

    return dict(
        Wq=t2(Wq), Wk=t2(Wk), Wqp=t2(Wqp),
        negblk=_bf(negblk.transpose(1, 0, 2)),    # [128, 2, 64]
        vkw=t2(vkw),                              # [128, 2, 320]
        Wo=_bf(Wo.transpose(1, 0, 2)),            # [128, 2, 256]
        mask8=_bf(mask8),                         # [64, 64]
    )


def _pack_host(inputs):
    i = {k: np.asarray(v, np.float32) for k, v in inputs.items()}
    for nm in ('ln1', 'ln2', 'ln3'):
        assert np.allclose(i[nm + '_g'], 1) and np.allclose(i[nm + '_b'], 0), \
            "non-identity LN gain/bias not supported by this kernel build"
    assert np.allclose(i['ffn_b1'], 0) and np.allclose(i['ffn_b2'], 0)

    m1 = _pack_performer(i['m1_qw'], i['m1_qb'], i['m1_kw'], i['m1_kb'],
                         i['m1_vw'], i['m1_vb'], i['m1_ow'], i['m1_ob'],
                         i['m1_feat'])
    m2 = _pack_performer(i['m2_qw'], i['m2_qb'], i['m2_kw'], i['m2_kb'],
                         i['m2_vw'], i['m2_vb'], i['m2_ow'], i['m2_ob'],
                         i['m2_feat'])

    w1 = i['ffn_w1']                       # [256, 1024]
    w2 = i['ffn_w2']                       # [1024, 256]
    W1 = _bf(w1.reshape(2, 128, DFF).transpose(1, 0, 2))       # [128, 2, 1024]
    W2 = _bf(w2.reshape(8, 128, D).transpose(1, 0, 2))         # [128, 8, 256]
    cf2 = _bf((i['ffn_b2'] - w2.sum(axis=0)).reshape(1, D))    # elu "-1" fold

    consts = {}
    for p, d in (('m1', m1), ('m2', m2)):
        for k, v in d.items():
            consts[f'{p}_{k}'] = v
    consts['W1'] = W1
    consts['W2'] = W2
    consts['cf2'] = cf2
    consts['I128'] = _bf(np.eye(128, dtype=np.float32))

    # per-core activations: own half FIRST, other half appended (kv is a
    # token sum, so order doesn't matter; q side reads tokens [0:S]).
    x = i['x']; enc = i['enc_output']
    xb = _bf(x); encb = _bf(enc)
    per_core = []
    for c in range(NCORES):
        b, hh = c // 2, c % 2
        own = slice(hh * S, (hh + 1) * S)
        oth = slice((1 - hh) * S, (2 - hh) * S)
        xperm = np.concatenate([xb[b, own], xb[b, oth]], axis=0)    # [N, D]
        eperm = np.concatenate([encb[b, own], encb[b, oth]], axis=0)
        per_core.append(dict(
            x_nat=np.ascontiguousarray(xb[b, own]),
            xT=np.ascontiguousarray(xperm.T),
            encT=np.ascontiguousarray(eperm.T),
            **consts,
        ))
    return per_core


# --------------------------------------------------------------------------
# Device program
# --------------------------------------------------------------------------

def _build_program(reps=1, single=False, phases=4):
    import concourse.bacc as bacc
    import concourse.tile as tile
    from concourse import mybir

    bf16, f32 = mybir.dt.bfloat16, mybir.dt.float32
    AF = mybir.ActivationFunctionType
    OP = mybir.AluOpType

    nc = bacc.Bacc("TRN2", target_bir_lowering=False, debug=False,
                   num_devices=1 if single else NCORES)

    din = {}
    def dram_in(name, shape):
        din[name] = nc.dram_tensor(name, list(shape), bf16,
                                   kind="ExternalInput")
        return din[name]

    x_nat_d = dram_in("x_nat", [S, D])
    xT_d = dram_in("xT", [D, N])
    encT_d = dram_in("encT", [D, N])
    for p in ('m1', 'm2'):
        dram_in(f'{p}_Wq', [128, 2, D]); dram_in(f'{p}_Wk', [128, 2, D])
        dram_in(f'{p}_Wqp', [128, 2, HM])
        dram_in(f'{p}_negblk', [128, 2, HM])
        dram_in(f'{p}_vkw', [128, 2, KP0 + HM])
        dram_in(f'{p}_Wo', [128, 2, D])
        dram_in(f'{p}_mask8', [HM, HM])
    dram_in('W1', [128, 2, DFF]); dram_in('W2', [128, 8, DFF // 4])
    dram_in('cf2', [1, D])
    dram_in('I128', [128, 128])
    y_d = nc.dram_tensor("y", [S, D], f32, kind="ExternalOutput")

    NSUB = S // 128  # 25

    with tile.TileContext(nc) as tc:
      from contextlib import ExitStack
      with ExitStack() as ctx:
        consts = ctx.enter_context(tc.tile_pool(name="consts", bufs=1))
        work = ctx.enter_context(tc.tile_pool(name="work", bufs=3))
        pkpool = ctx.enter_context(tc.tile_pool(name="pkpool", bufs=3))
        ffnpool = ctx.enter_context(tc.tile_pool(name="ffnpool", bufs=2))

        # ---- persistent SBUF tiles; loads issued in first-use order ----
        xT = consts.tile([128, 2, N], bf16, tag="xT_sb")
        xTv = xT_d.rearrange("(k p) t -> p k t", p=128)
        nc.sync.dma_start(out=xT[:, :, 0:1024], in_=xTv[:, :, 0:1024])
        cb = {}
        def cload(name):
            t = din[name]
            cb[name] = consts.tile(list(t.shape), bf16, tag=name,
                                   name=name + "_sb")
            nc.sync.dma_start(out=cb[name][:], in_=t[:])
        for nm in ('m1_Wk', 'm1_negblk', 'm1_vkw', 'm1_Wq', 'm1_Wqp'):
            cload(nm)
        nc.sync.dma_start(out=xT[:, :, 1024:N], in_=xTv[:, :, 1024:N])
        x_nat = consts.tile([128, NSUB, D], bf16, tag="xnat_sb")
        nc.sync.dma_start(out=x_nat[:],
                          in_=x_nat_d.rearrange("(n p) d -> p n d", p=128))
        cload('m1_mask8')
        encT = consts.tile([128, 2, N], bf16, tag="encT_sb")
        nc.sync.dma_start(out=encT[:],
                          in_=encT_d.rearrange("(k p) t -> p k t", p=128))
        for nm in ('m2_Wk', 'm2_negblk', 'm2_vkw', 'm2_mask8', 'I128',
                   'm1_Wo', 'm2_Wq', 'm2_Wqp', 'm2_Wo', 'W1', 'W2', 'cf2'):
            cload(nm)

        pq1 = consts.tile([HM, S], bf16, tag="pq1")
        pq2 = consts.tile([HM, S], bf16, tag="pq2")
        out1_nat = consts.tile([128, NSUB, D], bf16, tag="out1_nat")

        eps_t = consts.tile([128, 1], f32, tag="eps_t")
        nc.vector.memset(eps_t[:], EPS_LN)
        ones_row = consts.tile([1, 128], bf16, tag="ones_row")
        nc.vector.memset(ones_row[:], 1.0)
        # v3 double-buffer: [128, buf, pair-slot, 257]; ones col persistent
        v3db = consts.tile([128, 2, 2, VE], bf16, tag="v3db")
        nc.vector.memset(v3db[:, :, :, 256:257], 1.0)

        # ------------------------------------------------------------------
        def q_side(ps, srcT, pfx, pq_dst, c0, T):
            """pq_dst[:, c0:c0+T] = exp(q@feat - |q|^2/2) from srcT."""
            Wq, Wqp, negblk = cb[pfx + '_Wq'], cb[pfx + '_Wqp'], cb[pfx + '_negblk']
            q2t = work.tile([128, 2, 512], bf16, tag="q2t")
            for dt in range(2):
                q_ps = ps.tile([128, 512], f32, tag="proj", bufs=2)
                for kt in range(2):
                    nc.tensor.matmul(q_ps[:, :T], Wq[:, kt, 128*dt:128*dt+128],
                                     srcT[:, kt, c0:c0+T],
                                     start=(kt == 0), stop=(kt == 1))
                nc.scalar.activation(q2t[:, dt, :T], q_ps[:, :T], AF.Square)
            pq_ps = ps.tile([HM, 512], f32, tag="pqps", bufs=1)
            for kt in range(2):
                nc.tensor.matmul(pq_ps[:, :T], Wqp[:, kt, :], srcT[:, kt, c0:c0+T],
                                 start=(kt == 0), stop=False, skip_group_check=True)
            for dt in range(2):
                nc.tensor.matmul(pq_ps[:, :T], negblk[:, dt, :], q2t[:, dt, :T],
                                 start=False, stop=(dt == 1), skip_group_check=True)
            nc.scalar.activation(pq_dst[:, c0:c0+T], pq_ps[:, :T], AF.Exp)

        # ------------------------------------------------------------------
        _pair_n = [0]

        def kv_side(ps, srcT, pfx, kv_acc, c0, T, first, last):
            """Accumulate kv_acc [64, 257] over this chunk's tokens.

            v_ps [128, 2, 512] psum (2 banks, one per 128-token subchunk):
            cols 0:256 = v (h-major), 256:320 = kp = k@feat - |k|^2/2."""
            Wk, negblk, vkw = cb[pfx + '_Wk'], cb[pfx + '_negblk'], cb[pfx + '_vkw']
            k2sb = work.tile([128, 2, 512], bf16, tag="k2sb")
            for dt in range(2):
                k_ps = ps.tile([128, 512], f32, tag="proj", bufs=2)
                for kt in range(2):
                    nc.tensor.matmul(k_ps[:, :T], Wk[:, kt, 128*dt:128*dt+128],
                                     srcT[:, kt, c0:c0+T],
                                     start=(kt == 0), stop=(kt == 1))
                nc.vector.tensor_tensor(k2sb[:, dt, :T], k_ps[:, :T],
                                        k_ps[:, :T], OP.mult)
            npair = (T + 255) // 256
            for pr in range(npair):
                nsj = min(2, (T - 256 * pr + 127) // 128)
                v_ps = ps.tile([128, 2, 512], f32, tag="vps", bufs=2)
                for sj in range(nsj):
                    tok = slice(c0 + 256 * pr + 128 * sj,
                                c0 + 256 * pr + 128 * sj + 128)
                    lsl = slice(256 * pr + 128 * sj, 256 * pr + 128 * sj + 128)
                    for dt in range(2):
                        nc.tensor.matmul(v_ps[:, sj, 0:KP0 + HM],
                                         srcT[:, dt, tok], vkw[:, dt, :],
                                         start=(dt == 0), stop=False,
                                         skip_group_check=True)
                    for dt in range(2):
                        nc.tensor.matmul(v_ps[:, sj, KP0:KP0 + HM],
                                         k2sb[:, dt, lsl], negblk[:, dt, :],
                                         start=False, stop=(dt == 1),
                                         skip_group_check=True)
                bi = _pair_n[0] % 2; _pair_n[0] += 1
                pk = pkpool.tile([128, 2, HM], bf16, tag="pk")
                nc.scalar.activation(pk[:, 0:nsj, :], v_ps[:, 0:nsj, KP0:KP0 + HM],
                                     AF.Exp)
                nc.gpsimd.tensor_copy(v3db[:, bi, 0:nsj, 0:KP0],
                                      v_ps[:, 0:nsj, 0:KP0])
                for sj in range(nsj):
                    nc.tensor.matmul(kv_acc[:], pk[:, sj, :], v3db[:, bi, sj, :],
                                     start=(first and pr == 0 and sj == 0),
                                     stop=(last and pr == npair - 1 and sj == nsj - 1),
                                     skip_group_check=True)

        # ------------------------------------------------------------------
        def kv_finish(kv_acc, pfx):
            """kv psum -> sbuf bf16 + pbf (den matrix) build."""
            kv_sb = consts.tile([HM, VE], bf16, tag=pfx + "_kvsb",
                                name=pfx + "_kvsb")
            nc.vector.tensor_copy(kv_sb[:], kv_acc[:])
            pkcol = consts.tile([HM, 1], f32, tag=pfx + "_pkcol",
                                name=pfx + "_pkcol")
            nc.vector.tensor_copy(pkcol[:], kv_acc[:, 256:257])
            pbf = consts.tile([HM, HM], bf16, tag=pfx + "_pbf",
                              name=pfx + "_pbf")
            nc.vector.tensor_scalar(pbf[:], cb[pfx + '_mask8'][:], pkcol[:],
                                    None, OP.mult, OP.bypass)
            return kv_sb, pbf

        # ------------------------------------------------------------------
        def attn_apply(ps, pq_t, kv_sb, pbf, c0, T):
            """Return aT sbuf tile [128, 2, T] bf16 (feature-major attn out)."""
            denx_ps = ps.tile([HM, 512], f32, tag="den", bufs=1)
            nc.tensor.matmul(denx_ps[:, :T], pbf[:], pq_t[:, c0:c0+T],
                             start=True, stop=True)
            zr = work.tile([HM, 512], f32, tag="zr")
            nc.vector.reciprocal(zr[:, :T], denx_ps[:, :T])
            pqz = work.tile([HM, 512], bf16, tag="pqz")
            nc.vector.tensor_tensor(pqz[:, :T], pq_t[:, c0:c0+T],
                                    zr[:, :T], OP.mult)
            aT = work.tile([128, 2, 512], bf16, tag="aT")
            for g in range(2):
                aT_ps = ps.tile([128, 512], f32, tag="proj", bufs=2)
                nc.tensor.matmul(aT_ps[:, :T],
                                 kv_sb[32*g:32*g+32, 128*g:128*g+128],
                                 pqz[32*g:32*g+32, :T], start=True, stop=True)
                nc.scalar.activation(aT[:, g, :T], aT_ps[:, :T], AF.Copy)
            return aT

        # ------------------------------------------------------------------
        def ln_stats(r_f32, mvc, s_):
            st = work.tile([128, 6], f32, tag="lnst")
            nc.vector.bn_stats(out=st[:], in_=r_f32[:])
            nc.vector.bn_aggr(out=mvc[:, s_, :], in_=st[:])

        def ln_finish(mvc, ns):
            """rstd [128, ns] = exp(-0.5*ln(var+eps)) -- stays in exp set."""
            lnv = work.tile([128, 4], f32, tag="lnv")
            nc.scalar.activation(lnv[:, 0:ns], mvc[:, 0:ns, 1], AF.Ln,
                                 bias=eps_t[:])
            rstd = work.tile([128, 4], f32, tag="lnrstd")
            nc.scalar.activation(rstd[:, 0:ns], lnv[:, 0:ns], AF.Exp,
                                 scale=-0.5)
            return rstd

        def ln_norm(r_f32, mvc, rstd, s_, out_ap):
            nc.gpsimd.tensor_scalar(out_ap, r_f32[:], mvc[:, s_, 0:1],
                                    rstd[:, s_:s_+1], OP.subtract, OP.mult)

        # ------------------------------------------------------------------
        def trans_block(ps, src_nat, sub, dstT, c0, s_):
            """dstT[:, :, c0+128s : +128] = transpose of src_nat[:, sub, :]."""
            tr_ps = ps.tile([128, 2, 128], bf16, tag="trps", bufs=2)
            for h in range(2):
                nc.tensor.transpose(tr_ps[:, h, :],
                                    src_nat[:, sub, 128*h:128*h+128],
                                    cb['I128'][:])
            nc.gpsimd.tensor_copy(
                dstT[:, :, c0 + 128*s_:c0 + 128*s_ + 128], tr_ps[:])

        for _rep in range(reps):
            # ==============================================================
            # P1: m1 kv over full x (13 chunks) interleaved with m1 q_side
            #     over own tokens (7 chunks) -> pq1, kv1
            # ==============================================================
            with tc.tile_pool(name="ps1", bufs=1, space="PSUM") as ps:
                kv1_acc = ps.tile([HM, VE], f32, tag="kvacc", bufs=1)
                qi = 0
                for ci, (c0, T) in enumerate(CH_FULL):
                    kv_side(ps, xT, 'm1', kv1_acc, c0, T,
                            first=(ci == 0), last=(ci == len(CH_FULL) - 1))
                    if ci % 2 == 1 and qi < len(CH_OWN):
                        q_side(ps, xT, 'm1', pq1, *CH_OWN[qi]); qi += 1
                while qi < len(CH_OWN):
                    q_side(ps, xT, 'm1', pq1, *CH_OWN[qi]); qi += 1
                kv1_sb, pbf1 = kv_finish(kv1_acc, 'm1')

            if phases < 2:
                continue
            # ==============================================================
            # P2: m2 kv over full enc (13 chunks) -> kv2
            # ==============================================================
            with tc.tile_pool(name="ps2", bufs=1, space="PSUM") as ps:
                kv2_acc = ps.tile([HM, VE], f32, tag="kvacc", bufs=1)
                for ci, (c0, T) in enumerate(CH_FULL):
                    kv_side(ps, encT, 'm2', kv2_acc, c0, T,
                            first=(ci == 0), last=(ci == len(CH_FULL) - 1))
                kv2_sb, pbf2 = kv_finish(kv2_acc, 'm2')

            if phases < 3:
                continue
            # ==============================================================
            # P3: attn1 apply + LN1 -> out1 (nat + T) ; m2 q_side -> pq2
            # ==============================================================
            with tc.tile_pool(name="ps3", bufs=1, space="PSUM") as ps:
                for (c0, T) in CH_OWN:
                    aT = attn_apply(ps, pq1, kv1_sb, pbf1, c0, T)
                    out1T = work.tile([128, 2, 512], bf16, tag="outT",
                                      name="out1T")
                    mvc = work.tile([128, 4, 2], f32, tag="mvc", name="mvc")
                    ns = T // 128
                    r1s = work.tile([128, 4, D], f32, tag="r1s", name="r1s")
                    for s_ in range(ns):
                        sub = c0 // 128 + s_
                        sl = slice(128 * s_, 128 * s_ + 128)
                        o_ps = ps.tile([128, D], f32, tag="ops", bufs=2,
                                       name="o_ps")
                        for g in range(2):
                            nc.tensor.matmul(o_ps[:], aT[:, g, sl],
                                             cb['m1_Wo'][:, g, :],
                                             start=(g == 0), stop=(g == 1))
                        nc.vector.tensor_tensor(r1s[:, s_, :], o_ps[:],
                                                x_nat[:, sub, :], OP.add)
                        ln_stats(r1s[:, s_, :], mvc, s_)
                    rstd = ln_finish(mvc, ns)
                    for s_ in range(ns):
                        sub = c0 // 128 + s_
                        ln_norm(r1s[:, s_, :], mvc, rstd, s_,
                                out1_nat[:, sub, :])
                        trans_block(ps, out1_nat, sub, out1T, 0, s_)
                    # m2 q_side on chunk-local out1T (c0 -> 0)
                    Wq, Wqp, negblk = cb['m2_Wq'], cb['m2_Wqp'], cb['m2_negblk']
                    q2t = work.tile([128, 2, 512], bf16, tag="q2t")
                    for dt in range(2):
                        q_ps = ps.tile([128, 512], f32, tag="proj", bufs=2)
                        for kt in range(2):
                            nc.tensor.matmul(q_ps[:, :T],
                                             Wq[:, kt, 128*dt:128*dt+128],
                                             out1T[:, kt, :T],
                                             start=(kt == 0), stop=(kt == 1))
                        nc.scalar.activation(q2t[:, dt, :T], q_ps[:, :T],
                                             AF.Square)
                    pq_ps = ps.tile([HM, 512], f32, tag="pqps", bufs=1)
                    for kt in range(2):
                        nc.tensor.matmul(pq_ps[:, :T], Wqp[:, kt, :],
                                         out1T[:, kt, :T],
                                         start=(kt == 0), stop=False,
                                         skip_group_check=True)
                    for dt in range(2):
                        nc.tensor.matmul(pq_ps[:, :T], negblk[:, dt, :],
                                         q2t[:, dt, :T],
                                         start=False, stop=(dt == 1),
                                         skip_group_check=True)
                    nc.scalar.activation(pq2[:, c0:c0+T], pq_ps[:, :T], AF.Exp)

            if phases < 4:
                continue
            # ==============================================================
            # P4: attn2 apply + LN2 -> out2 ; FFN ; LN3 -> y
            # ==============================================================
            with tc.tile_pool(name="ps4", bufs=1, space="PSUM") as ps:
                for (c0, T) in CH_OWN:
                    aT2 = attn_apply(ps, pq2, kv2_sb, pbf2, c0, T)
                    out2_nat = work.tile([128, 4, D], bf16, tag="out2_nat",
                                         name="out2_nat")
                    out2T = work.tile([128, 2, 512], bf16, tag="outT",
                                      name="out2T")
                    mvc2 = work.tile([128, 4, 2], f32, tag="mvc", name="mvc2")
                    ns = T // 128
                    r2s = work.tile([128, 4, D], f32, tag="r1s", name="r2s")
                    for s_ in range(ns):
                        sub = c0 // 128 + s_
                        sl = slice(128 * s_, 128 * s_ + 128)
                        o_ps = ps.tile([128, D], f32, tag="ops", bufs=2,
                                       name="o_ps")
                        for g in range(2):
                            nc.tensor.matmul(o_ps[:], aT2[:, g, sl],
                                             cb['m2_Wo'][:, g, :],
                                             start=(g == 0), stop=(g == 1))
                        nc.vector.tensor_tensor(r2s[:, s_, :], o_ps[:],
                                                out1_nat[:, sub, :], OP.add)
                        ln_stats(r2s[:, s_, :], mvc2, s_)
                    rstd2 = ln_finish(mvc2, ns)
                    for s_ in range(ns):
                        ln_norm(r2s[:, s_, :], mvc2, rstd2, s_,
                                out2_nat[:, s_, :])
                        trans_block(ps, out2_nat, s_, out2T, 0, s_)
                    # FFN on out2T
                    hs = ffnpool.tile([128, 8, 512], bf16, tag="hs", name="hs")
                    for f in range(8):
                        h_ps = ps.tile([128, 512], f32, tag="proj", bufs=2,
                                       name="h_ps")
                        for kt in range(2):
                            nc.tensor.matmul(h_ps[:, :T],
                                             cb['W1'][:, kt, 128*f:128*f+128],
                                             out2T[:, kt, :T],
                                             start=(kt == 0), stop=(kt == 1))
                        ex = work.tile([128, 512], f32, tag="ffnex", name="ex")
                        nc.scalar.activation(ex[:, :T], h_ps[:, :T], AF.Exp)
                        em = work.tile([128, 512], bf16, tag="ffnem", name="em")
                        nc.gpsimd.tensor_scalar_min(em[:, :T], ex[:, :T], 1.0)
                        nc.vector.scalar_tensor_tensor(hs[:, f, :T], h_ps[:, :T],
                                                       0.0, em[:, :T],
                                                       OP.max, OP.add)
                    mvc3 = work.tile([128, 4, 2], f32, tag="mvc", name="mvc3")
                    r3s = work.tile([128, 4, D], f32, tag="r1s", name="r3s")
                    for s_ in range(ns):
                        sl = slice(128 * s_, 128 * s_ + 128)
                        f_ps = ps.tile([128, D], f32, tag="ops", bufs=2,
                                       name="f_ps")
                        nc.tensor.matmul(f_ps[:], ones_row[:], cb['cf2'][:],
                                         start=True, stop=False,
                                         skip_group_check=True)
                        for kt in range(8):
                            nc.tensor.matmul(f_ps[:], hs[:, kt, sl],
                                             cb['W2'][:, kt, :],
                                             start=False, stop=(kt == 7),
                                             skip_group_check=True)
                        nc.vector.tensor_tensor(r3s[:, s_, :], f_ps[:],
                                                out2_nat[:, s_, :], OP.add)
                        ln_stats(r3s[:, s_, :], mvc3, s_)
                    rstd3 = ln_finish(mvc3, ns)
                    for s_ in range(ns):
                        sub = c0 // 128 + s_
                        o3 = work.tile([128, D], f32, tag="o3", name="o3")
                        ln_norm(r3s[:, s_, :], mvc3, rstd3, s_, o3[:])
                        nc.sync.dma_start(
                            out=y_d.rearrange("(n p) d -> p n d", p=128)[:, sub, :],
                            in_=o3[:])

    nc.compile()
    return nc


_prog_cache = {}


def _get_program(reps=1):
    key = ('nc', reps)
    if key not in _prog_cache:
        _prog_cache[key] = _build_program(reps)
    return _prog_cache[key]


def kernel(**inputs):
    from concourse.bass_utils import run_bass_kernel_spmd
    per_core = _pack_host(inputs)
    nc = _get_program()
    res = run_bass_kernel_spmd(nc, per_core, core_ids=list(range(NCORES)))
    out = np.empty((B, N, D), np.float32)
    for c in range(NCORES):
        b, hh = c // 2, c % 2
        out[b, hh * S:(hh + 1) * S, :] = res.results[c]["y"]
    return out


if __name__ == "__main__":
    import reference as R
    inp = R.setup_inputs()
    ref = np.asarray(R.reference(**inp))
    got = kernel(**{k: np.asarray(v) for k, v in inp.items()})
    rel = np.linalg.norm(got - ref) / np.linalg.norm(ref)
    print("Relative error:", rel)
    print("max abs err:", np.abs(got - ref).max())


# revision 7
# speedup vs baseline: 1.2046x; 1.2046x over previous
"""Trainium2 Bass kernel for nn_DecoderLayer (Performer/FAVOR+ decoder layer).

Problem: B=4, N=6400, D=256, H=8, HD=32, DFF=1024, M(feat)=8.
  out = LN3(FFN(LN2(CrossPerf(LN1(SelfPerf(x)+x)) + ...)) + ...)

v2 design (no collectives, no DMA transposes, single act-table set):
  - 8 cores; core c handles batch c//2, token half c%2 (3200 own tokens).
  - kv (the token reduction of linear attention) is computed over the FULL
    6400 tokens per core (x / enc of its batch), so no AllReduce is needed.
    Host permutes tokens so each core's own half comes first; kv is
    order-invariant, the q side just reads tokens [0:3200].
  - kv accumulator [64, 257] f32 in PSUM: cols 0:256 = per-head v (h-major),
    col 256 = sum_t phi(k).  The block-diagonal attention operands are then
    plain slices kv[32g:32g+32, 128g:128g+128] -- no scatter, no DRAM trip.
  - Layout transposes (out1/out2 natural -> feature-major for the next
    matmul) are PE transposes via an identity matrix, not DMA transposes.
  - LayerNorm rstd = exp(-0.5*ln(var+eps)): ln/exp/square/copy all live in
    one activation-function table set, so no act-table reloads.
All matmul I/O bf16 (fp32 PSUM accumulation); residual stream bf16; final
LN3 output written f32.
"""

import numpy as np
import ml_dtypes

B, N, D, H, HD, DFF, M = 4, 6400, 256, 8, 32, 1024, 8
NCORES = 8
S = B * N // NCORES          # 3200 own tokens per core
EPS_LN = 1e-6
SCALE = float(HD) ** -0.25   # split softmax scale on q and k
HM = H * M                   # 64
VE = 257                     # kv cols: 256 v (h-major) + 1 pksum col
KP0 = 256                    # kp feature region start in v_ps psum

CH_FULL = [(i * 512, 512) for i in range(12)] + [(6144, 256)]
CH_OWN = [(i * 512, 512) for i in range(6)] + [(3072, 128)]

_BF = ml_dtypes.bfloat16


def _bf(a):
    return np.ascontiguousarray(np.asarray(a, np.float32)).astype(_BF)


# --------------------------------------------------------------------------
# Host-side weight packing
# --------------------------------------------------------------------------

def _pack_performer(qw, qb, kw, kb, vw, vb, ow, ob, feat):
    """Pack one performer's weights into the device layouts (all bf16)."""
    qw = np.asarray(qw, np.float32); kw = np.asarray(kw, np.float32)
    vw = np.asarray(vw, np.float32); ow = np.asarray(ow, np.float32)
    feat = np.asarray(feat, np.float32)
    assert np.allclose(np.asarray(qb), 0) and np.allclose(np.asarray(kb), 0), \
        "nonzero q/k bias not supported by this kernel build"
    assert np.allclose(np.asarray(vb), 0) and np.allclose(np.asarray(ob), 0), \
        "nonzero v/o bias not supported by this kernel build"

    Wq = (SCALE * qw).reshape(D, D)          # [d, (h,hd)]
    Wk = (SCALE * kw).reshape(D, D)
    # W_qp[d, 8h+j] = sum_hd s*qw[d,h,hd] * feat[hd,j]
    Wqp = np.einsum('dhk,km->dhm', SCALE * qw, feat).reshape(D, HM)
    Wkp = np.einsum('dhk,km->dhm', SCALE * kw, feat).reshape(D, HM)

    # blockdiag -0.5 reducer over a 128-row douttile (4 heads x 32 hd)
    negblk = np.zeros((2, 128, HM), np.float32)
    for g in range(2):
        for lh in range(4):
            h = 4 * g + lh
            negblk[g, 32 * lh:32 * lh + 32, 8 * h:8 * h + 8] = -0.5

    # vkw [256, 320]: cols 0:256 v-projection (h-major), 256:320 Wkp
    vkw = np.concatenate([vw.reshape(D, D), Wkp], axis=1)

    Wo = np.zeros((2, 128, D), np.float32)   # out-proj rhs per group g
    for g in range(2):
        for lh in range(4):
            h = 4 * g + lh
            Wo[g, 32 * lh:32 * lh + 32, :] = ow[h, :, :]

    mask8 = np.zeros((HM, HM), np.float32)   # [8h+j, 8h'+j'] = [h==h']
    for h in range(H):
        mask8[8*h:8*h+8, 8*h:8*h+8] = 1.0

    def t2(a):  # [D, C] -> [128, 2, C] with [i, kt, c] = a[128*kt + i, c]
        return _bf(a.reshape(2, 128, -1).transpose(1, 0, 2))

    return dict(
        Wq=t2(Wq), Wk=t2(Wk), Wqp=t2(Wqp),
        negblk=_bf(negblk.transpose(1, 0, 2)),    # [128, 2, 64]
        vkw=t2(vkw),                              # [128, 2, 320]
        Wo=_bf(Wo.transpose(1, 0, 2)),            # [128, 2, 256]
        mask8=_bf(mask8),                         # [64, 64]
    )


def _pack_host(inputs):
    i = {k: np.asarray(v, np.float32) for k, v in inputs.items()}
    for nm in ('ln1', 'ln2', 'ln3'):
        assert np.allclose(i[nm + '_g'], 1) and np.allclose(i[nm + '_b'], 0), \
            "non-identity LN gain/bias not supported by this kernel build"
    assert np.allclose(i['ffn_b1'], 0) and np.allclose(i['ffn_b2'], 0)

    m1 = _pack_performer(i['m1_qw'], i['m1_qb'], i['m1_kw'], i['m1_kb'],
                         i['m1_vw'], i['m1_vb'], i['m1_ow'], i['m1_ob'],
                         i['m1_feat'])
    m2 = _pack_performer(i['m2_qw'], i['m2_qb'], i['m2_kw'], i['m2_kb'],
                         i['m2_vw'], i['m2_vb'], i['m2_ow'], i['m2_ob'],
                         i['m2_feat'])

    w1 = i['ffn_w1']                       # [256, 1024]
    w2 = i['ffn_w2']                       # [1024, 256]
    W1 = _bf(w1.reshape(2, 128, DFF).transpose(1, 0, 2))       # [128, 2, 1024]
    W2 = _bf(w2.reshape(8, 128, D).transpose(1, 0, 2))         # [128, 8, 256]
    cf2 = _bf((i['ffn_b2'] - w2.sum(axis=0)).reshape(1, D))    # elu "-1" fold

    consts = {}
    for p, d in (('m1', m1), ('m2', m2)):
        for k, v in d.items():
            consts[f'{p}_{k}'] = v
    consts['W1'] = W1
    consts['W2'] = W2
    consts['cf2'] = cf2
    consts['I128'] = _bf(np.eye(128, dtype=np.float32))

    # per-core activations: own half FIRST, other half appended (kv is a
    # token sum, so order doesn't matter; q side reads tokens [0:S]).
    x = i['x']; enc = i['enc_output']
    xb = _bf(x); encb = _bf(enc)
    per_core = []
    for c in range(NCORES):
        b, hh = c // 2, c % 2
        own = slice(hh * S, (hh + 1) * S)
        oth = slice((1 - hh) * S, (2 - hh) * S)
        xperm = np.concatenate([xb[b, own], xb[b, oth]], axis=0)    # [N, D]
        eperm = np.concatenate([encb[b, own], encb[b, oth]], axis=0)
        per_core.append(dict(
            x_nat=np.ascontiguousarray(xb[b, own]),
            xT=np.ascontiguousarray(xperm.T),
            encT=np.ascontiguousarray(eperm.T),
            **consts,
        ))
    return per_core


# --------------------------------------------------------------------------
# Device program
# --------------------------------------------------------------------------

def _build_program(reps=1, single=False, phases=4):
    import concourse.bacc as bacc
    import concourse.tile as tile
    from concourse import mybir

    bf16, f32 = mybir.dt.bfloat16, mybir.dt.float32
    AF = mybir.ActivationFunctionType
    OP = mybir.AluOpType

    nc = bacc.Bacc("TRN2", target_bir_lowering=False, debug=False,
                   num_devices=1 if single else NCORES)

    din = {}
    def dram_in(name, shape):
        din[name] = nc.dram_tensor(name, list(shape), bf16,
                                   kind="ExternalInput")
        return din[name]

    x_nat_d = dram_in("x_nat", [S, D])
    xT_d = dram_in("xT", [D, N])
    encT_d = dram_in("encT", [D, N])
    for p in ('m1', 'm2'):
        dram_in(f'{p}_Wq', [128, 2, D]); dram_in(f'{p}_Wk', [128, 2, D])
        dram_in(f'{p}_Wqp', [128, 2, HM])
        dram_in(f'{p}_negblk', [128, 2, HM])
        dram_in(f'{p}_vkw', [128, 2, KP0 + HM])
        dram_in(f'{p}_Wo', [128, 2, D])
        dram_in(f'{p}_mask8', [HM, HM])
    dram_in('W1', [128, 2, DFF]); dram_in('W2', [128, 8, DFF // 4])
    dram_in('cf2', [1, D])
    dram_in('I128', [128, 128])
    y_d = nc.dram_tensor("y", [S, D], f32, kind="ExternalOutput")

    NSUB = S // 128  # 25

    with tile.TileContext(nc) as tc:
      from contextlib import ExitStack
      with ExitStack() as ctx:
        consts = ctx.enter_context(tc.tile_pool(name="consts", bufs=1))
        work = ctx.enter_context(tc.tile_pool(name="work", bufs=3))
        pkpool = ctx.enter_context(tc.tile_pool(name="pkpool", bufs=3))
        ffnpool = ctx.enter_context(tc.tile_pool(name="ffnpool", bufs=2))

        # ---- persistent SBUF tiles; loads issued in first-use order ----
        xT = consts.tile([128, 2, N], bf16, tag="xT_sb")
        xTv = xT_d.rearrange("(k p) t -> p k t", p=128)
        nc.sync.dma_start(out=xT[:, :, 0:1024], in_=xTv[:, :, 0:1024])
        cb = {}
        def cload(name):
            t = din[name]
            cb[name] = consts.tile(list(t.shape), bf16, tag=name,
                                   name=name + "_sb")
            nc.sync.dma_start(out=cb[name][:], in_=t[:])
        for nm in ('m1_Wk', 'm1_negblk', 'm1_vkw', 'm1_Wq', 'm1_Wqp'):
            cload(nm)
        nc.sync.dma_start(out=xT[:, :, 1024:N], in_=xTv[:, :, 1024:N])
        x_nat = consts.tile([128, NSUB, D], bf16, tag="xnat_sb")
        nc.sync.dma_start(out=x_nat[:],
                          in_=x_nat_d.rearrange("(n p) d -> p n d", p=128))
        cload('m1_mask8')
        encT = consts.tile([128, 2, N], bf16, tag="encT_sb")
        nc.sync.dma_start(out=encT[:],
                          in_=encT_d.rearrange("(k p) t -> p k t", p=128))
        for nm in ('m2_Wk', 'm2_negblk', 'm2_vkw', 'm2_mask8', 'I128',
                   'm1_Wo', 'm2_Wq', 'm2_Wqp', 'm2_Wo', 'W1', 'W2', 'cf2'):
            cload(nm)

        pq1 = consts.tile([HM, S], bf16, tag="pq1")
        pq2 = consts.tile([HM, S], bf16, tag="pq2")
        out1_nat = consts.tile([128, NSUB, D], bf16, tag="out1_nat")
        rall = consts.tile([128, NSUB, D], f32, tag="rall")
        mvc = consts.tile([128, NSUB, 2], f32, tag="mvc")
        rstd = consts.tile([128, NSUB], f32, tag="rstd")

        eps_t = consts.tile([128, 1], f32, tag="eps_t")
        nc.vector.memset(eps_t[:], EPS_LN)
        ones_row = consts.tile([1, 128], bf16, tag="ones_row")
        nc.vector.memset(ones_row[:], 1.0)
        # v3 double-buffer: [128, buf, pair-slot, 257]; ones col persistent
        v3db = consts.tile([128, 2, 2, VE], bf16, tag="v3db")
        nc.vector.memset(v3db[:, :, :, 256:257], 1.0)

        # ------------------------------------------------------------------
        def q_side(ps, srcT, pfx, pq_dst, c0, T, local=False):
            """pq_dst[:, c0:c0+T] = exp(q@feat - |q|^2/2) from srcT.
            local=True: srcT is a chunk-local [128, 2, T] tile."""
            Wq, Wqp, negblk = cb[pfx + '_Wq'], cb[pfx + '_Wqp'], cb[pfx + '_negblk']
            s0 = 0 if local else c0
            q2t = work.tile([128, 2, 512], bf16, tag="q2t")
            for dt in range(2):
                q_ps = ps.tile([128, 512], f32, tag="proj", bufs=2)
                for kt in range(2):
                    nc.tensor.matmul(q_ps[:, :T], Wq[:, kt, 128*dt:128*dt+128],
                                     srcT[:, kt, s0:s0+T],
                                     start=(kt == 0), stop=(kt == 1))
                nc.scalar.activation(q2t[:, dt, :T], q_ps[:, :T], AF.Square)
            pq_ps = ps.tile([HM, 512], f32, tag="pqps", bufs=1)
            for kt in range(2):
                nc.tensor.matmul(pq_ps[:, :T], Wqp[:, kt, :], srcT[:, kt, s0:s0+T],
                                 start=(kt == 0), stop=False, skip_group_check=True)
            for dt in range(2):
                nc.tensor.matmul(pq_ps[:, :T], negblk[:, dt, :], q2t[:, dt, :T],
                                 start=False, stop=(dt == 1), skip_group_check=True)
            nc.scalar.activation(pq_dst[:, c0:c0+T], pq_ps[:, :T], AF.Exp)

        # ------------------------------------------------------------------
        _pair_n = [0]

        def kv_side(ps, srcT, pfx, kv_acc, c0, T, first, last):
            """Accumulate kv_acc [64, 257] over this chunk's tokens."""
            Wk, negblk, vkw = cb[pfx + '_Wk'], cb[pfx + '_negblk'], cb[pfx + '_vkw']
            k2sb = work.tile([128, 2, 512], bf16, tag="k2sb")
            for dt in range(2):
                k_ps = ps.tile([128, 512], f32, tag="proj", bufs=2)
                for kt in range(2):
                    nc.tensor.matmul(k_ps[:, :T], Wk[:, kt, 128*dt:128*dt+128],
                                     srcT[:, kt, c0:c0+T],
                                     start=(kt == 0), stop=(kt == 1))
                nc.vector.tensor_tensor(k2sb[:, dt, :T], k_ps[:, :T],
                                        k_ps[:, :T], OP.mult)
            npair = (T + 255) // 256
            vps, pks, bis = [], [], []
            for pr in range(npair):
                nsj = min(2, (T - 256 * pr + 127) // 128)
                v_ps = ps.tile([128, 2, 512], f32, tag="vps", bufs=2)
                vps.append((v_ps, nsj))
                # v projections first (no k2 dependency)
                for sj in range(nsj):
                    tok = slice(c0 + 256 * pr + 128 * sj,
                                c0 + 256 * pr + 128 * sj + 128)
                    for dt in range(2):
                        nc.tensor.matmul(v_ps[:, sj, 0:KP0 + HM],
                                         srcT[:, dt, tok], vkw[:, dt, :],
                                         start=(dt == 0), stop=False,
                                         skip_group_check=True)
            for pr in range(npair):
                v_ps, nsj = vps[pr]
                for sj in range(nsj):
                    lsl = slice(256 * pr + 128 * sj, 256 * pr + 128 * sj + 128)
                    for dt in range(2):
                        nc.tensor.matmul(v_ps[:, sj, KP0:KP0 + HM],
                                         k2sb[:, dt, lsl], negblk[:, dt, :],
                                         start=False, stop=(dt == 1),
                                         skip_group_check=True)
            for pr in range(npair):
                v_ps, nsj = vps[pr]
                bi = _pair_n[0] % 2; _pair_n[0] += 1
                pk = pkpool.tile([128, 2, HM], bf16, tag="pk")
                nc.scalar.activation(pk[:, 0:nsj, :],
                                     v_ps[:, 0:nsj, KP0:KP0 + HM], AF.Exp)
                nc.gpsimd.tensor_copy(v3db[:, bi, 0:nsj, 0:KP0],
                                      v_ps[:, 0:nsj, 0:KP0])
                pks.append(pk); bis.append(bi)
            for pr in range(npair):
                v_ps, nsj = vps[pr]
                for sj in range(nsj):
                    nc.tensor.matmul(kv_acc[:], pks[pr][:, sj, :],
                                     v3db[:, bis[pr], sj, :],
                                     start=(first and pr == 0 and sj == 0),
                                     stop=(last and pr == npair - 1 and sj == nsj - 1),
                                     skip_group_check=True)

        # ------------------------------------------------------------------
        def kv_finish(kv_acc, pfx):
            kv_sb = consts.tile([HM, VE], bf16, tag=pfx + "_kvsb",
                                name=pfx + "_kvsb")
            nc.vector.tensor_copy(kv_sb[:], kv_acc[:])
            pkcol = consts.tile([HM, 1], f32, tag=pfx + "_pkcol",
                                name=pfx + "_pkcol")
            nc.vector.tensor_copy(pkcol[:], kv_acc[:, 256:257])
            pbf = consts.tile([HM, HM], bf16, tag=pfx + "_pbf",
                              name=pfx + "_pbf")
            nc.vector.tensor_scalar(pbf[:], cb[pfx + '_mask8'][:], pkcol[:],
                                    None, OP.mult, OP.bypass)
            return kv_sb, pbf

        # ------------------------------------------------------------------
        def attn_apply(ps, pq_t, kv_sb, pbf, c0, T):
            """Return aT sbuf tile [128, 2, T] bf16 (feature-major attn out)."""
            denx_ps = ps.tile([HM, 512], f32, tag="den", bufs=1)
            nc.tensor.matmul(denx_ps[:, :T], pbf[:], pq_t[:, c0:c0+T],
                             start=True, stop=True)
            zr = work.tile([HM, 512], f32, tag="zr")
            nc.vector.reciprocal(zr[:, :T], denx_ps[:, :T])
            pqz = work.tile([HM, 512], bf16, tag="pqz")
            nc.vector.tensor_tensor(pqz[:, :T], pq_t[:, c0:c0+T],
                                    zr[:, :T], OP.mult)
            aT = work.tile([128, 2, 512], bf16, tag="aT")
            for g in range(2):
                aT_ps = ps.tile([128, 512], f32, tag="proj", bufs=2)
                nc.tensor.matmul(aT_ps[:, :T],
                                 kv_sb[32*g:32*g+32, 128*g:128*g+128],
                                 pqz[32*g:32*g+32, :T], start=True, stop=True)
                nc.scalar.activation(aT[:, g, :T], aT_ps[:, :T], AF.Copy)
            return aT

        # ------------------------------------------------------------------
        def out_stats(ps, aT, Wo, res_nat, c0, T):
            """o = aT.T@Wo + res -> rall; bn stats -> mvc (phase-wide)."""
            ns = T // 128
            for s_ in range(ns):
                sub = c0 // 128 + s_
                sl = slice(128 * s_, 128 * s_ + 128)
                o_ps = ps.tile([128, D], f32, tag="ops", bufs=2)
                for g in range(2):
                    nc.tensor.matmul(o_ps[:], aT[:, g, sl], Wo[:, g, :],
                                     start=(g == 0), stop=(g == 1))
                nc.vector.tensor_tensor(rall[:, sub, :], o_ps[:],
                                        res_nat[:, sub, :], OP.add)
                st = work.tile([128, 6], f32, tag="lnst")
                nc.vector.bn_stats(out=st[:], in_=rall[:, sub, :])
                nc.vector.bn_aggr(out=mvc[:, sub, :], in_=st[:])

        def rstd_batch():
            """rstd[:, all] = 1/sqrt(var+eps) -- ONE sqrt + ONE reciprocal."""
            std = work.tile([128, NSUB], f32, tag="lnstd")
            nc.scalar.activation(std[:], mvc[:, :, 1], AF.Sqrt, bias=eps_t[:])
            nc.vector.reciprocal(rstd[:], std[:])

        def ln_norm(sub, out_ap):
            nc.gpsimd.tensor_scalar(out_ap, rall[:, sub, :], mvc[:, sub, 0:1],
                                    rstd[:, sub:sub+1], OP.subtract, OP.mult)

        def trans_block(ps, src_nat, sub, dstT, s_):
            """dstT[:, :, 128s : +128] = transpose of src_nat[:, sub, :]."""
            tr_ps = ps.tile([128, 2, 128], bf16, tag="trps", bufs=2)
            for h in range(2):
                nc.tensor.transpose(tr_ps[:, h, :],
                                    src_nat[:, sub, 128*h:128*h+128],
                                    cb['I128'][:])
            nc.gpsimd.tensor_copy(dstT[:, :, 128*s_:128*s_+128], tr_ps[:])

        for _rep in range(reps):
            # ==============================================================
            # P1: m1 kv over full x (13 chunks) + m1 q_side own (7 chunks)
            # ==============================================================
            with tc.tile_pool(name="ps1", bufs=1, space="PSUM") as ps:
                kv1_acc = ps.tile([HM, VE], f32, tag="kvacc", bufs=1)
                qi = 0
                for ci, (c0, T) in enumerate(CH_FULL):
                    kv_side(ps, xT, 'm1', kv1_acc, c0, T,
                            first=(ci == 0), last=(ci == len(CH_FULL) - 1))
                    if ci % 2 == 1 and qi < len(CH_OWN):
                        q_side(ps, xT, 'm1', pq1, *CH_OWN[qi]); qi += 1
                while qi < len(CH_OWN):
                    q_side(ps, xT, 'm1', pq1, *CH_OWN[qi]); qi += 1
                kv1_sb, pbf1 = kv_finish(kv1_acc, 'm1')

            if phases < 2:
                continue
            # ==============================================================
            # P2: m2 kv over full enc (13 chunks) -> kv2
            # ==============================================================
            with tc.tile_pool(name="ps2", bufs=1, space="PSUM") as ps:
                kv2_acc = ps.tile([HM, VE], f32, tag="kvacc", bufs=1)
                for ci, (c0, T) in enumerate(CH_FULL):
                    kv_side(ps, encT, 'm2', kv2_acc, c0, T,
                            first=(ci == 0), last=(ci == len(CH_FULL) - 1))
                kv2_sb, pbf2 = kv_finish(kv2_acc, 'm2')

            if phases < 3:
                continue
            # ==============================================================
            # P3: attn1+LN1 -> out1 (nat+T per chunk); m2 q_side -> pq2
            # ==============================================================
            with tc.tile_pool(name="ps3", bufs=1, space="PSUM") as ps:
                # P3a: attn apply + residual + stats (head/tail pipelined)
                prev = None
                for (c0, T) in CH_OWN:
                    aT = attn_apply(ps, pq1, kv1_sb, pbf1, c0, T)
                    if prev is not None:
                        out_stats(ps, *prev)
                    prev = (aT, cb['m1_Wo'], x_nat, c0, T)
                out_stats(ps, *prev)
                # P3b: batched rstd
                rstd_batch()
                # P3c: norm -> out1_nat; transpose -> out1T; q2 side -> pq2
                for (c0, T) in CH_OWN:
                    out1T = work.tile([128, 2, 512], bf16, tag="outT",
                                      name="out1T")
                    for s_ in range(T // 128):
                        sub = c0 // 128 + s_
                        ln_norm(sub, out1_nat[:, sub, :])
                        trans_block(ps, out1_nat, sub, out1T, s_)
                    q_side(ps, out1T, 'm2', pq2, c0, T, local=True)

            if phases < 4:
                continue
            # ==============================================================
            # P4: attn2+LN2 -> out2; FFN; LN3 -> y
            # ==============================================================
            with tc.tile_pool(name="ps4", bufs=1, space="PSUM") as ps:
                # P4a: attn2 + residual + stats
                prev = None
                for (c0, T) in CH_OWN:
                    aT2 = attn_apply(ps, pq2, kv2_sb, pbf2, c0, T)
                    if prev is not None:
                        out_stats(ps, *prev)
                    prev = (aT2, cb['m2_Wo'], out1_nat, c0, T)
                out_stats(ps, *prev)
                # P4b: batched rstd2
                rstd_batch()

                # P4c: norm2 -> out2_nat/out2T; FFN h+ELU; f2+res+stats
                def p4c_head(c0, T):
                    out2_nat = work.tile([128, 4, D], bf16, tag="out2_nat",
                                         name="out2_nat")
                    out2T = work.tile([128, 2, 512], bf16, tag="outT",
                                      name="out2T")
                    ns = T // 128
                    for s_ in range(ns):
                        sub = c0 // 128 + s_
                        ln_norm(sub, out2_nat[:, s_, :])
                        trans_block(ps, out2_nat, s_, out2T, s_)
                    hs = ffnpool.tile([128, 8, 512], bf16, tag="hs", name="hs")
                    for f in range(8):
                        h_ps = ps.tile([128, 512], f32, tag="proj", bufs=2,
                                       name="h_ps")
                        for kt in range(2):
                            nc.tensor.matmul(h_ps[:, :T],
                                             cb['W1'][:, kt, 128*f:128*f+128],
                                             out2T[:, kt, :T],
                                             start=(kt == 0), stop=(kt == 1))
                        ex = work.tile([128, 512], f32, tag="ffnex", name="ex")
                        nc.scalar.activation(ex[:, :T], h_ps[:, :T], AF.Exp)
                        em = work.tile([128, 512], bf16, tag="ffnem", name="em")
                        nc.gpsimd.tensor_scalar_min(em[:, :T], ex[:, :T], 1.0)
                        nc.vector.scalar_tensor_tensor(hs[:, f, :T],
                                                       h_ps[:, :T], 0.0,
                                                       em[:, :T], OP.max, OP.add)
                    return out2_nat, hs

                def p4c_tail(out2_nat, hs, c0, T):
                    ns = T // 128
                    for s_ in range(ns):
                        sub = c0 // 128 + s_
                        sl = slice(128 * s_, 128 * s_ + 128)
                        f_ps = ps.tile([128, D], f32, tag="ops", bufs=2,
                                       name="f_ps")
                        nc.tensor.matmul(f_ps[:], ones_row[:], cb['cf2'][:],
                                         start=True, stop=False,
                                         skip_group_check=True)
                        for kt in range(8):
                            nc.tensor.matmul(f_ps[:], hs[:, kt, sl],
                                             cb['W2'][:, kt, :],
                                             start=False, stop=(kt == 7),
                                             skip_group_check=True)
                        nc.vector.tensor_tensor(rall[:, sub, :], f_ps[:],
                                                out2_nat[:, s_, :], OP.add)
                        st = work.tile([128, 6], f32, tag="lnst")
                        nc.vector.bn_stats(out=st[:], in_=rall[:, sub, :])
                        nc.vector.bn_aggr(out=mvc[:, sub, :], in_=st[:])

                prev = None
                for (c0, T) in CH_OWN:
                    cur = p4c_head(c0, T)
                    if prev is not None:
                        p4c_tail(*prev)
                    prev = (*cur, c0, T)
                p4c_tail(*prev)

                # P4d: batched rstd3
                rstd_batch()
                # P4e: norm3 -> y DMA out (pairs of subtiles)
                for sp in range(0, NSUB, 2):
                    nsp = min(2, NSUB - sp)
                    o3 = work.tile([128, 2, D], f32, tag="o3", name="o3")
                    for s_ in range(nsp):
                        ln_norm(sp + s_, o3[:, s_, :])
                    nc.sync.dma_start(
                        out=y_d.rearrange("(n p) d -> p n d", p=128)[:, sp:sp+nsp, :],
                        in_=o3[:, 0:nsp, :])

    nc.compile()
    return nc


_prog_cache = {}


def _get_program(reps=1):
    key = ('nc', reps)
    if key not in _prog_cache:
        _prog_cache[key] = _build_program(reps)
    return _prog_cache[key]


def kernel(**inputs):
    from concourse.bass_utils import run_bass_kernel_spmd
    per_core = _pack_host(inputs)
    nc = _get_program()
    res = run_bass_kernel_spmd(nc, per_core, core_ids=list(range(NCORES)))
    out = np.empty((B, N, D), np.float32)
    for c in range(NCORES):
        b, hh = c // 2, c % 2
        out[b, hh * S:(hh + 1) * S, :] = res.results[c]["y"]
    return out


if __name__ == "__main__":
    import reference as R
    inp = R.setup_inputs()
    ref = np.asarray(R.reference(**inp))
    got = kernel(**{k: np.asarray(v) for k, v in inp.items()})
    rel = np.linalg.norm(got - ref) / np.linalg.norm(ref)
    print("Relative error:", rel)
    print("max abs err:", np.abs(got - ref).max())


# revision 13
# speedup vs baseline: 2.3805x; 1.9762x over previous
"""Trainium2 Bass kernel for nn_DecoderLayer (Performer/FAVOR+ decoder layer).

Problem: B=4, N=6400, D=256, H=8, HD=32, DFF=1024, M(feat)=8.
  out = LN3(FFN(LN2(CrossPerf(LN1(SelfPerf(x)+x)) + ...)) + ...)

v2 design (no collectives, no DMA transposes, single act-table set):
  - 8 cores; core c handles batch c//2, token half c%2 (3200 own tokens).
  - kv (the token reduction of linear attention) is computed over the FULL
    6400 tokens per core (x / enc of its batch), so no AllReduce is needed.
    Host permutes tokens so each core's own half comes first; kv is
    order-invariant, the q side just reads tokens [0:3200].
  - kv accumulator [64, 257] f32 in PSUM: cols 0:256 = per-head v (h-major),
    col 256 = sum_t phi(k).  The block-diagonal attention operands are then
    plain slices kv[32g:32g+32, 128g:128g+128] -- no scatter, no DRAM trip.
  - Layout transposes (out1/out2 natural -> feature-major for the next
    matmul) are PE transposes via an identity matrix, not DMA transposes.
  - LayerNorm rstd = exp(-0.5*ln(var+eps)): ln/exp/square/copy all live in
    one activation-function table set, so no act-table reloads.
All matmul I/O bf16 (fp32 PSUM accumulation); residual stream bf16; final
LN3 output written f32.
"""

import numpy as np
import ml_dtypes

B, N, D, H, HD, DFF, M = 4, 6400, 256, 8, 32, 1024, 8
NCORES = 8
S = B * N // NCORES          # 3200 own tokens per core
EPS_LN = 1e-6
SCALE = float(HD) ** -0.25   # split softmax scale on q and k
HM = H * M                   # 64
VE = 257                     # kv cols: 256 v (h-major) + 1 pksum col
KP0 = 256                    # kp feature region start in v_ps psum

CH_FULL = [(i * 512, 512) for i in range(12)] + [(6144, 256)]
CH_OWN = [(i * 512, 512) for i in range(6)] + [(3072, 128)]

_BF = ml_dtypes.bfloat16


def _bf(a):
    return np.ascontiguousarray(np.asarray(a, np.float32)).astype(_BF)


# --------------------------------------------------------------------------
# Host-side weight packing
# --------------------------------------------------------------------------

def _pack_performer(qw, qb, kw, kb, vw, vb, ow, ob, feat):
    """Pack one performer's weights into the device layouts (all bf16)."""
    qw = np.asarray(qw, np.float32); kw = np.asarray(kw, np.float32)
    vw = np.asarray(vw, np.float32); ow = np.asarray(ow, np.float32)
    feat = np.asarray(feat, np.float32)
    assert np.allclose(np.asarray(qb), 0) and np.allclose(np.asarray(kb), 0), \
        "nonzero q/k bias not supported by this kernel build"
    assert np.allclose(np.asarray(vb), 0) and np.allclose(np.asarray(ob), 0), \
        "nonzero v/o bias not supported by this kernel build"

    Wq = (SCALE * qw).reshape(D, D)          # [d, (h,hd)]
    Wk = (SCALE * kw).reshape(D, D)
    # W_qp[d, 8h+j] = sum_hd s*qw[d,h,hd] * feat[hd,j]
    Wqp = np.einsum('dhk,km->dhm', SCALE * qw, feat).reshape(D, HM)
    Wkp = np.einsum('dhk,km->dhm', SCALE * kw, feat).reshape(D, HM)

    # blockdiag -0.5 reducer over a 128-row douttile (4 heads x 32 hd)
    negblk = np.zeros((2, 128, HM), np.float32)
    for g in range(2):
        for lh in range(4):
            h = 4 * g + lh
            negblk[g, 32 * lh:32 * lh + 32, 8 * h:8 * h + 8] = -0.5

    # vkw [256, 320]: cols 0:256 v-projection (h-major), 256:320 Wkp
    vkw = np.concatenate([vw.reshape(D, D), Wkp], axis=1)

    Wo = np.zeros((2, 128, D), np.float32)   # out-proj rhs per group g
    for g in range(2):
        for lh in range(4):
            h = 4 * g + lh
            Wo[g, 32 * lh:32 * lh + 32, :] = ow[h, :, :]

    mask8 = np.zeros((HM, HM), np.float32)   # [8h+j, 8h'+j'] = [h==h']
    for h in range(H):
        mask8[8*h:8*h+8, 8*h:8*h+8] = 1.0

    def t2(a):  # [D, C] -> [128, 2, C] with [i, kt, c] = a[128*kt + i, c]
        return _bf(a.reshape(2, 128, -1).transpose(1, 0, 2))

    return dict(
        Wq=t2(Wq), Wk=t2(Wk), Wqp=t2(Wqp),
        negblk=_bf(negblk.transpose(1, 0, 2)),    # [128, 2, 64]
        vkw=t2(vkw),                              # [128, 2, 320]
        Wo=_bf(Wo.transpose(1, 0, 2)),            # [128, 2, 256]
        mask8=_bf(mask8),                         # [64, 64]
    )


def _pack_host(inputs):
    i = {k: np.asarray(v, np.float32) for k, v in inputs.items()}
    for nm in ('ln1', 'ln2', 'ln3'):
        assert np.allclose(i[nm + '_g'], 1) and np.allclose(i[nm + '_b'], 0), \
            "non-identity LN gain/bias not supported by this kernel build"
    assert np.allclose(i['ffn_b1'], 0) and np.allclose(i['ffn_b2'], 0)

    m1 = _pack_performer(i['m1_qw'], i['m1_qb'], i['m1_kw'], i['m1_kb'],
                         i['m1_vw'], i['m1_vb'], i['m1_ow'], i['m1_ob'],
                         i['m1_feat'])
    m2 = _pack_performer(i['m2_qw'], i['m2_qb'], i['m2_kw'], i['m2_kb'],
                         i['m2_vw'], i['m2_vb'], i['m2_ow'], i['m2_ob'],
                         i['m2_feat'])

    w1 = i['ffn_w1']                       # [256, 1024]
    w2 = i['ffn_w2']                       # [1024, 256]
    W1 = _bf(w1.reshape(2, 128, DFF).transpose(1, 0, 2))       # [128, 2, 1024]
    W2 = _bf(w2.reshape(8, 128, D).transpose(1, 0, 2))         # [128, 8, 256]
    cf2 = _bf((i['ffn_b2'] - w2.sum(axis=0)).reshape(1, D))    # elu "-1" fold

    consts = {}
    for p, d in (('m1', m1), ('m2', m2)):
        for k, v in d.items():
            consts[f'{p}_{k}'] = v
    consts['W1'] = W1
    consts['W2'] = W2
    consts['cf2'] = cf2
    consts['I128'] = _bf(np.eye(128, dtype=np.float32))

    # per-core activations: own half FIRST, other half appended (kv is a
    # token sum, so order doesn't matter; q side reads tokens [0:S]).
    x = i['x']; enc = i['enc_output']
    xb = _bf(x); encb = _bf(enc)
    per_core = []
    for c in range(NCORES):
        b, hh = c // 2, c % 2
        own = slice(hh * S, (hh + 1) * S)
        oth = slice((1 - hh) * S, (2 - hh) * S)
        xperm = np.concatenate([xb[b, own], xb[b, oth]], axis=0)    # [N, D]
        eperm = np.concatenate([encb[b, own], encb[b, oth]], axis=0)
        per_core.append(dict(
            x_nat=np.ascontiguousarray(xb[b, own]),
            xT=np.ascontiguousarray(xperm.T),
            encT=np.ascontiguousarray(eperm.T),
            **consts,
        ))
    return per_core


# --------------------------------------------------------------------------
# Device program
# --------------------------------------------------------------------------

def _build_program(reps=1, single=False, phases=4):
    import concourse.bacc as bacc
    import concourse.tile as tile
    from concourse import mybir

    bf16, f32 = mybir.dt.bfloat16, mybir.dt.float32
    AF = mybir.ActivationFunctionType
    OP = mybir.AluOpType

    nc = bacc.Bacc("TRN2", target_bir_lowering=False, debug=False,
                   num_devices=1 if single else NCORES)

    din = {}
    def dram_in(name, shape):
        din[name] = nc.dram_tensor(name, list(shape), bf16,
                                   kind="ExternalInput")
        return din[name]

    x_nat_d = dram_in("x_nat", [S, D])
    xT_d = dram_in("xT", [D, N])
    encT_d = dram_in("encT", [D, N])
    for p in ('m1', 'm2'):
        dram_in(f'{p}_Wq', [128, 2, D]); dram_in(f'{p}_Wk', [128, 2, D])
        dram_in(f'{p}_Wqp', [128, 2, HM])
        dram_in(f'{p}_negblk', [128, 2, HM])
        dram_in(f'{p}_vkw', [128, 2, KP0 + HM])
        dram_in(f'{p}_Wo', [128, 2, D])
        dram_in(f'{p}_mask8', [HM, HM])
    dram_in('W1', [128, 2, DFF]); dram_in('W2', [128, 8, DFF // 4])
    dram_in('cf2', [1, D])
    dram_in('I128', [128, 128])
    y_d = nc.dram_tensor("y", [S, D], f32, kind="ExternalOutput")

    NSUB = S // 128  # 25

    with tile.TileContext(nc) as tc:
      from contextlib import ExitStack
      with ExitStack() as ctx:
        consts = ctx.enter_context(tc.tile_pool(name="consts", bufs=1))
        work = ctx.enter_context(tc.tile_pool(name="work", bufs=3))
        pkpool = ctx.enter_context(tc.tile_pool(name="pkpool", bufs=3))
        ffnpool = ctx.enter_context(tc.tile_pool(name="ffnpool", bufs=2))

        # ---- persistent SBUF tiles; loads issued in first-use order ----
        xT = consts.tile([128, 2, N], bf16, tag="xT_sb")
        xTv = xT_d.rearrange("(k p) t -> p k t", p=128)
        nc.sync.dma_start(out=xT[:, :, 0:1024], in_=xTv[:, :, 0:1024])
        cb = {}
        def cload(name):
            t = din[name]
            cb[name] = consts.tile(list(t.shape), bf16, tag=name,
                                   name=name + "_sb")
            nc.sync.dma_start(out=cb[name][:], in_=t[:])
        for nm in ('m1_Wk', 'm1_negblk', 'm1_vkw', 'm1_Wq', 'm1_Wqp'):
            cload(nm)
        nc.sync.dma_start(out=xT[:, :, 1024:N], in_=xTv[:, :, 1024:N])
        x_nat = consts.tile([128, NSUB, D], bf16, tag="xnat_sb")
        nc.sync.dma_start(out=x_nat[:],
                          in_=x_nat_d.rearrange("(n p) d -> p n d", p=128))
        cload('m1_mask8')
        encT = consts.tile([128, 2, N], bf16, tag="encT_sb")
        nc.sync.dma_start(out=encT[:],
                          in_=encT_d.rearrange("(k p) t -> p k t", p=128))
        for nm in ('m2_Wk', 'm2_negblk', 'm2_vkw', 'm2_mask8', 'I128',
                   'm1_Wo', 'm2_Wq', 'm2_Wqp', 'm2_Wo', 'W1', 'W2', 'cf2'):
            cload(nm)

        pq1 = consts.tile([HM, S], bf16, tag="pq1")
        pq2 = consts.tile([HM, S], bf16, tag="pq2")
        out1_nat = consts.tile([128, NSUB, D], bf16, tag="out1_nat")
        rall = consts.tile([128, NSUB, D], f32, tag="rall")
        mvc = consts.tile([128, NSUB, 2], f32, tag="mvc")
        rstd = consts.tile([128, NSUB], f32, tag="rstd")

        eps_t = consts.tile([128, 1], f32, tag="eps_t")
        nc.vector.memset(eps_t[:], EPS_LN)
        ones_row = consts.tile([1, 128], bf16, tag="ones_row")
        nc.vector.memset(ones_row[:], 1.0)
        # v3 ring buffer: [128, buf, pair-slot, 257]; ones col persistent
        NV3 = 4
        v3db = consts.tile([128, NV3, 2, VE], bf16, tag="v3db")
        nc.vector.memset(v3db[:, :, :, 256:257], 1.0)

        # ------------------------------------------------------------------
        def q_side(ps, srcT, pfx, pq_dst, c0, T, local=False):
            """pq_dst[:, c0:c0+T] = exp(q@feat - |q|^2/2) from srcT.
            local=True: srcT is a chunk-local [128, 2, T] tile."""
            Wq, Wqp, negblk = cb[pfx + '_Wq'], cb[pfx + '_Wqp'], cb[pfx + '_negblk']
            s0 = 0 if local else c0
            q2t = work.tile([128, 2, 512], bf16, tag="q2t")
            for dt in range(2):
                q_ps = ps.tile([128, 512], f32, tag="proj", bufs=2)
                for kt in range(2):
                    nc.tensor.matmul(q_ps[:, :T], Wq[:, kt, 128*dt:128*dt+128],
                                     srcT[:, kt, s0:s0+T],
                                     start=(kt == 0), stop=(kt == 1))
                nc.scalar.activation(q2t[:, dt, :T], q_ps[:, :T], AF.Square)
            pq_ps = ps.tile([HM, 512], f32, tag="pqps", bufs=1)
            for kt in range(2):
                nc.tensor.matmul(pq_ps[:, :T], Wqp[:, kt, :], srcT[:, kt, s0:s0+T],
                                 start=(kt == 0), stop=False, skip_group_check=True)
            for dt in range(2):
                nc.tensor.matmul(pq_ps[:, :T], negblk[:, dt, :], q2t[:, dt, :T],
                                 start=False, stop=(dt == 1), skip_group_check=True)
            nc.scalar.activation(pq_dst[:, c0:c0+T], pq_ps[:, :T], AF.Exp)

        # ------------------------------------------------------------------
        _pair_n = [0]

        def kv_side(ps, srcT, pfx, kv_acc, c0, T, first, last):
            """Accumulate kv_acc [64, 257] over this chunk's tokens."""
            Wk, negblk, vkw = cb[pfx + '_Wk'], cb[pfx + '_negblk'], cb[pfx + '_vkw']
            k2sb = work.tile([128, 2, 512], bf16, tag="k2sb")
            for dt in range(2):
                k_ps = ps.tile([128, 512], f32, tag="proj", bufs=2)
                for kt in range(2):
                    nc.tensor.matmul(k_ps[:, :T], Wk[:, kt, 128*dt:128*dt+128],
                                     srcT[:, kt, c0:c0+T],
                                     start=(kt == 0), stop=(kt == 1))
                nc.vector.tensor_tensor(k2sb[:, dt, :T], k_ps[:, :T],
                                        k_ps[:, :T], OP.mult)
            npair = (T + 255) // 256
            vps, pks, bis = [], [], []
            for pr in range(npair):
                nsj = min(2, (T - 256 * pr + 127) // 128)
                v_ps = ps.tile([128, 2, 512], f32, tag="vps", bufs=2)
                vps.append((v_ps, nsj))
                # v projections first (no k2 dependency)
                for sj in range(nsj):
                    tok = slice(c0 + 256 * pr + 128 * sj,
                                c0 + 256 * pr + 128 * sj + 128)
                    for dt in range(2):
                        nc.tensor.matmul(v_ps[:, sj, 0:KP0 + HM],
                                         srcT[:, dt, tok], vkw[:, dt, :],
                                         start=(dt == 0), stop=False,
                                         skip_group_check=True)
            for pr in range(npair):
                v_ps, nsj = vps[pr]
                for sj in range(nsj):
                    lsl = slice(256 * pr + 128 * sj, 256 * pr + 128 * sj + 128)
                    for dt in range(2):
                        nc.tensor.matmul(v_ps[:, sj, KP0:KP0 + HM],
                                         k2sb[:, dt, lsl], negblk[:, dt, :],
                                         start=False, stop=(dt == 1),
                                         skip_group_check=True)
            for pr in range(npair):
                v_ps, nsj = vps[pr]
                bi = _pair_n[0] % NV3; _pair_n[0] += 1
                pk = pkpool.tile([128, 2, HM], bf16, tag="pk")
                nc.scalar.activation(pk[:, 0:nsj, :],
                                     v_ps[:, 0:nsj, KP0:KP0 + HM], AF.Exp)
                nc.vector.tensor_copy(v3db[:, bi, 0:nsj, 0:KP0],
                                      v_ps[:, 0:nsj, 0:KP0])
                pks.append(pk); bis.append(bi)
            for pr in range(npair):
                v_ps, nsj = vps[pr]
                for sj in range(nsj):
                    nc.tensor.matmul(kv_acc[:], pks[pr][:, sj, :],
                                     v3db[:, bis[pr], sj, :],
                                     start=(first and pr == 0 and sj == 0),
                                     stop=(last and pr == npair - 1 and sj == nsj - 1),
                                     skip_group_check=True)

        # ------------------------------------------------------------------
        def kv_finish(kv_acc, pfx):
            kv_sb = consts.tile([HM, VE], bf16, tag=pfx + "_kvsb",
                                name=pfx + "_kvsb")
            nc.vector.tensor_copy(kv_sb[:], kv_acc[:])
            pkcol = consts.tile([HM, 1], f32, tag=pfx + "_pkcol",
                                name=pfx + "_pkcol")
            nc.vector.tensor_copy(pkcol[:], kv_acc[:, 256:257])
            pbf = consts.tile([HM, HM], bf16, tag=pfx + "_pbf",
                              name=pfx + "_pbf")
            nc.vector.tensor_scalar(pbf[:], cb[pfx + '_mask8'][:], pkcol[:],
                                    None, OP.mult, OP.bypass)
            return kv_sb, pbf

        # ------------------------------------------------------------------
        def attn_apply(ps, pq_t, kv_sb, pbf, c0, T):
            """Return aT sbuf tile [128, 2, T] bf16 (feature-major attn out)."""
            denx_ps = ps.tile([HM, 512], f32, tag="den", bufs=1)
            nc.tensor.matmul(denx_ps[:, :T], pbf[:], pq_t[:, c0:c0+T],
                             start=True, stop=True)
            zr = work.tile([HM, 512], f32, tag="zr")
            nc.vector.reciprocal(zr[:, :T], denx_ps[:, :T])
            pqz = work.tile([HM, 512], bf16, tag="pqz")
            nc.vector.tensor_tensor(pqz[:, :T], pq_t[:, c0:c0+T],
                                    zr[:, :T], OP.mult)
            aT = work.tile([128, 2, 512], bf16, tag="aT")
            for g in range(2):
                aT_ps = ps.tile([128, 512], f32, tag="proj", bufs=2)
                nc.tensor.matmul(aT_ps[:, :T],
                                 kv_sb[32*g:32*g+32, 128*g:128*g+128],
                                 pqz[32*g:32*g+32, :T], start=True, stop=True)
                nc.scalar.activation(aT[:, g, :T], aT_ps[:, :T], AF.Copy)
            return aT

        # ------------------------------------------------------------------
        def out_stats(ps, aT, Wo, res_nat, c0, T):
            """o = aT.T@Wo + res -> rall; bn stats -> mvc (phase-wide)."""
            ns = T // 128
            for s_ in range(ns):
                sub = c0 // 128 + s_
                sl = slice(128 * s_, 128 * s_ + 128)
                o_ps = ps.tile([128, D], f32, tag="ops", bufs=2)
                for g in range(2):
                    nc.tensor.matmul(o_ps[:], aT[:, g, sl], Wo[:, g, :],
                                     start=(g == 0), stop=(g == 1))
                nc.vector.tensor_tensor(rall[:, sub, :], o_ps[:],
                                        res_nat[:, sub, :], OP.add)
                st = work.tile([128, 6], f32, tag="lnst")
                nc.vector.bn_stats(out=st[:], in_=rall[:, sub, :])
                nc.vector.bn_aggr(out=mvc[:, sub, :], in_=st[:])

        def rstd_batch():
            """rstd[:, all] = 1/sqrt(var+eps) -- ONE sqrt + ONE reciprocal."""
            std = work.tile([128, NSUB], f32, tag="lnstd")
            nc.scalar.activation(std[:], mvc[:, :, 1], AF.Sqrt, bias=eps_t[:])
            nc.vector.reciprocal(rstd[:], std[:])

        def ln_norm(sub, out_ap):
            nc.gpsimd.tensor_scalar(out_ap, rall[:, sub, :], mvc[:, sub, 0:1],
                                    rstd[:, sub:sub+1], OP.subtract, OP.mult)

        def trans_block(ps, src_nat, sub, dstT, s_):
            """dstT[:, :, 128s : +128] = transpose of src_nat[:, sub, :]."""
            tr_ps = ps.tile([128, 2, 128], bf16, tag="trps", bufs=2)
            for h in range(2):
                nc.tensor.transpose(tr_ps[:, h, :],
                                    src_nat[:, sub, 128*h:128*h+128],
                                    cb['I128'][:])
            nc.gpsimd.tensor_copy(dstT[:, :, 128*s_:128*s_+128], tr_ps[:])

        for _rep in range(reps):
            # ==============================================================
            # P1: m1 kv over full x (13 chunks) + m1 q_side own (7 chunks)
            # ==============================================================
            with tc.tile_pool(name="ps1", bufs=1, space="PSUM") as ps:
                kv1_acc = ps.tile([HM, VE], f32, tag="kvacc", bufs=1)
                qi = 0
                for ci, (c0, T) in enumerate(CH_FULL):
                    kv_side(ps, xT, 'm1', kv1_acc, c0, T,
                            first=(ci == 0), last=(ci == len(CH_FULL) - 1))
                    if ci % 2 == 1 and qi < len(CH_OWN):
                        q_side(ps, xT, 'm1', pq1, *CH_OWN[qi]); qi += 1
                while qi < len(CH_OWN):
                    q_side(ps, xT, 'm1', pq1, *CH_OWN[qi]); qi += 1
                kv1_sb, pbf1 = kv_finish(kv1_acc, 'm1')

            if phases < 2:
                continue
            # ==============================================================
            # P2: m2 kv over full enc (13 chunks) -> kv2
            # ==============================================================
            with tc.tile_pool(name="ps2", bufs=1, space="PSUM") as ps:
                kv2_acc = ps.tile([HM, VE], f32, tag="kvacc", bufs=1)
                for ci, (c0, T) in enumerate(CH_FULL):
                    kv_side(ps, encT, 'm2', kv2_acc, c0, T,
                            first=(ci == 0), last=(ci == len(CH_FULL) - 1))
                kv2_sb, pbf2 = kv_finish(kv2_acc, 'm2')

            if phases < 3:
                continue
            # ==============================================================
            # P3: attn1+LN1 -> out1 (nat+T per chunk); m2 q_side -> pq2
            # ==============================================================
            with tc.tile_pool(name="ps3a", bufs=1, space="PSUM") as ps:
                # P3a: attn apply + residual + stats (head/tail pipelined)
                prev = None
                for (c0, T) in CH_OWN:
                    aT = attn_apply(ps, pq1, kv1_sb, pbf1, c0, T)
                    if prev is not None:
                        out_stats(ps, *prev)
                    prev = (aT, cb['m1_Wo'], x_nat, c0, T)
                out_stats(ps, *prev)
                # P3b: batched rstd
                rstd_batch()
            with tc.tile_pool(name="ps3c", bufs=1, space="PSUM") as ps:
                # P3c: norm -> out1_nat; transpose -> out1T; q2 side -> pq2
                for (c0, T) in CH_OWN:
                    out1T = work.tile([128, 2, 512], bf16, tag="outT",
                                      name="out1T")
                    for s_ in range(T // 128):
                        sub = c0 // 128 + s_
                        ln_norm(sub, out1_nat[:, sub, :])
                        trans_block(ps, out1_nat, sub, out1T, s_)
                    q_side(ps, out1T, 'm2', pq2, c0, T, local=True)

            if phases < 4:
                continue
            # ==============================================================
            # P4: attn2+LN2 -> out2; FFN; LN3 -> y
            # ==============================================================
            with tc.tile_pool(name="ps4a", bufs=1, space="PSUM") as ps:
                # P4a: attn2 + residual + stats
                prev = None
                for (c0, T) in CH_OWN:
                    aT2 = attn_apply(ps, pq2, kv2_sb, pbf2, c0, T)
                    if prev is not None:
                        out_stats(ps, *prev)
                    prev = (aT2, cb['m2_Wo'], out1_nat, c0, T)
                out_stats(ps, *prev)
                # P4b: batched rstd2
                rstd_batch()

            with tc.tile_pool(name="ps4c", bufs=1, space="PSUM") as ps:
                # P4c: norm2 -> out2_nat/out2T; FFN h+ELU; f2+res+stats
                def p4c_head(c0, T):
                    out2_nat = work.tile([128, 4, D], bf16, tag="out2_nat",
                                         name="out2_nat")
                    out2T = work.tile([128, 2, 512], bf16, tag="outT",
                                      name="out2T")
                    ns = T // 128
                    for s_ in range(ns):
                        sub = c0 // 128 + s_
                        ln_norm(sub, out2_nat[:, s_, :])
                        trans_block(ps, out2_nat, s_, out2T, s_)
                    hs = ffnpool.tile([128, 8, 512], bf16, tag="hs", name="hs")
                    for f in range(8):
                        h_ps = ps.tile([128, 512], f32, tag="hps", bufs=4,
                                       name="h_ps")
                        for kt in range(2):
                            nc.tensor.matmul(h_ps[:, :T],
                                             cb['W1'][:, kt, 128*f:128*f+128],
                                             out2T[:, kt, :T],
                                             start=(kt == 0), stop=(kt == 1))
                        ex = work.tile([128, 512], bf16, tag="ffnex", name="ex")
                        nc.scalar.activation(ex[:, :T], h_ps[:, :T], AF.Exp)
                        em = work.tile([128, 512], bf16, tag="ffnem", name="em")
                        nc.gpsimd.tensor_scalar_min(em[:, :T], ex[:, :T], 1.0)
                        nc.vector.scalar_tensor_tensor(hs[:, f, :T],
                                                       h_ps[:, :T], 0.0,
                                                       em[:, :T], OP.max, OP.add)
                    return out2_nat, hs

                def p4c_tail(out2_nat, hs, c0, T):
                    ns = T // 128
                    for s_ in range(ns):
                        sub = c0 // 128 + s_
                        sl = slice(128 * s_, 128 * s_ + 128)
                        f_ps = ps.tile([128, D], f32, tag="ops", bufs=2,
                                       name="f_ps")
                        nc.tensor.matmul(f_ps[:], ones_row[:], cb['cf2'][:],
                                         start=True, stop=False,
                                         skip_group_check=True)
                        for kt in range(8):
                            nc.tensor.matmul(f_ps[:], hs[:, kt, sl],
                                             cb['W2'][:, kt, :],
                                             start=False, stop=(kt == 7),
                                             skip_group_check=True)
                        nc.vector.tensor_tensor(rall[:, sub, :], f_ps[:],
                                                out2_nat[:, s_, :], OP.add)
                        st = work.tile([128, 6], f32, tag="lnst")
                        nc.vector.bn_stats(out=st[:], in_=rall[:, sub, :])
                        nc.vector.bn_aggr(out=mvc[:, sub, :], in_=st[:])

                prev = None
                for (c0, T) in CH_OWN:
                    cur = p4c_head(c0, T)
                    if prev is not None:
                        p4c_tail(*prev)
                    prev = (*cur, c0, T)
                p4c_tail(*prev)

                # P4d: batched rstd3
                rstd_batch()
                # P4e: norm3 -> y DMA out (pairs of subtiles)
                for sp in range(0, NSUB, 2):
                    nsp = min(2, NSUB - sp)
                    o3 = work.tile([128, 2, D], f32, tag="o3", name="o3")
                    for s_ in range(nsp):
                        ln_norm(sp + s_, o3[:, s_, :])
                    nc.sync.dma_start(
                        out=y_d.rearrange("(n p) d -> p n d", p=128)[:, sp:sp+nsp, :],
                        in_=o3[:, 0:nsp, :])

    nc.compile()
    return nc


_prog_cache = {}


def _get_program(reps=1):
    key = ('nc', reps)
    if key not in _prog_cache:
        _prog_cache[key] = _build_program(reps)
    return _prog_cache[key]


def kernel(**inputs):
    from concourse.bass_utils import run_bass_kernel_spmd
    per_core = _pack_host(inputs)
    nc = _get_program()
    res = run_bass_kernel_spmd(nc, per_core, core_ids=list(range(NCORES)))
    out = np.empty((B, N, D), np.float32)
    for c in range(NCORES):
        b, hh = c // 2, c % 2
        out[b, hh * S:(hh + 1) * S, :] = res.results[c]["y"]
    return out


if __name__ == "__main__":
    import reference as R
    inp = R.setup_inputs()
    ref = np.asarray(R.reference(**inp))
    got = kernel(**{k: np.asarray(v) for k, v in inp.items()})
    rel = np.linalg.norm(got - ref) / np.linalg.norm(ref)
    print("Relative error:", rel)
    print("max abs err:", np.abs(got - ref).max())
